# revision 5
# baseline (speedup 1.0000x reference)
"""Trainium2 Bass kernel for nn_DiffusionModel (auction-matched flow targets).

Self-contained: accepts FULL inputs (cloud [16,2048,3], noise [16,2048,3],
t [16]), shards batch over 8 NeuronCores (2 samples per core), runs the full
5-iteration bijective auction per sample on device, returns [2,16,2048,3].

v2: candidate-cache fast path. Iteration 0 performs the full dense scan
(PE matmul V0 build fused with DVE Max/MaxIndex top-8 per 128-row group).
Iterations 1-4 revalidate each row's top-2 against the frozen iteration-0
top-8 candidate set: since all prices are >= 0 and non-candidate columns
satisfy V0 <= bound (8th-best V0), a row's current top-2 provably stays
within its candidates whenever cand-m2 >= bound. Groups with any invalid
row are rescanned under a data-dependent tc.If branch (DVE-only body).
Candidate prices are fetched per partition with gpsimd ap_gather (block-
shared indices) + a static diagonal-lane mask reduction. Output alignment
also uses ap_gather instead of dense mask-reduce passes.
"""
import numpy as np

P = 128
N = 2048
NG = 16          # row groups per sample (NG * P = N rows)
D = 3
SPC = 2          # samples per core
EPS = 1e-3
NCORES = 8
REPEAT = 1       # benchmark knob: repeat the whole per-core pipeline
FASTPATH = True  # False: rescan every group every iteration (debug)


def _build_program():
    import concourse.bass as bass
    import concourse.tile as tile
    from concourse import bacc, mybir, bass_isa

    fp32 = mybir.dt.float32
    u16 = mybir.dt.uint16
    i16 = mybir.dt.int16
    i32 = mybir.dt.int32
    OP = mybir.AluOpType
    AX = mybir.AxisListType
    AF = mybir.ActivationFunctionType

    nc = bacc.Bacc("TRN2", target_bir_lowering=False, debug=False,
                   enable_asserts=False)

    # ---- DRAM I/O ----
    noiseT_d = nc.dram_tensor("noiseT", [SPC, 3, N], fp32, kind="ExternalInput")
    cloudT_d = nc.dram_tensor("cloudT", [SPC, 3, N], fp32, kind="ExternalInput")
    cloudR_d = nc.dram_tensor("cloudR", [SPC, P, NG * D], fp32, kind="ExternalInput")
    noiseR_d = nc.dram_tensor("noiseR", [SPC, P, NG * D], fp32, kind="ExternalInput")
    tv_d = nc.dram_tensor("tv", [SPC, 1], fp32, kind="ExternalInput")
    ltc_d = nc.dram_tensor("ltc", [P, NG * NG], u16, kind="ExternalInput")
    ecs_d = nc.dram_tensor("ecs", [35, 3 * P], fp32, kind="ExternalInput")
    onesrow_d = nc.dram_tensor("onesrow", [1, N], fp32, kind="ExternalInput")
    diag_d = nc.dram_tensor("diag", [P, 16], fp32, kind="ExternalInput")
    out_d = nc.dram_tensor("out", [SPC, 2, P, NG * D], fp32, kind="ExternalOutput")

    with tile.TileContext(nc) as tc:
        with (
            tc.tile_pool(name="v0", bufs=1) as v0pool,
            tc.tile_pool(name="dense", bufs=1) as dp,
            tc.tile_pool(name="small", bufs=1) as sp,
            tc.tile_pool(name="psA", bufs=6, space="PSUM") as psA,
            tc.tile_pool(name="psB", bufs=1, space="PSUM") as psB,
        ):
            # ---- constants (persist across both samples) ----
            LTC = sp.tile([P, NG * NG], u16, tag="ltc")
            nc.sync.dma_start(LTC[:], ltc_d.ap())
            DIAG = sp.tile([P, 16], fp32, tag="diag")
            nc.sync.dma_start(DIAG[:], diag_d.ap())
            ones_128x1 = sp.tile([P, 1], fp32, tag="ones_128x1")
            onescol = sp.tile([67, 1], fp32, tag="onescol")
            MINUS1 = sp.tile([P, NG], fp32, tag="minus1")
            NEGB = sp.tile([P, NG * 8], fp32, tag="negb")
            U16MAX = sp.tile([P, NG * 8], u16, tag="u16max")
            ECS = sp.tile([35, 3 * P], fp32, tag="ecs")
            nc.sync.dma_start(ECS[:], ecs_d.ap())
            nc.vector.memset(ones_128x1[:], 1.0)
            nc.vector.memset(onescol[:], 0.0)
            nc.vector.memset(onescol[64:67, :], 1.0)
            nc.vector.memset(MINUS1[:], -1.0)
            nc.vector.memset(NEGB[:], -3.0e38)
            nc.vector.memset(U16MAX[:], 65535)

            for s in [s for _ in range(REPEAT) for s in range(SPC)]:
                # ================= prep =================
                strip1 = sp.tile([P, N], fp32, tag="strip1")
                strip2 = sp.tile([P, N], fp32, tag=f"strip2_{s % 2}")
                nT = strip1[32:36, :]
                csq = strip1[64:67, :]
                X04 = strip2[32:36, :]
                cR = sp.tile([P, NG * D], fp32, tag="cR")
                nR = sp.tile([P, NG * D], fp32, tag="nR")
                nc.sync.dma_start(nT[0:3, :], noiseT_d.ap()[s])
                nc.sync.dma_start(nT[3:4, :], onesrow_d.ap())
                nc.sync.dma_start(csq[:], cloudT_d.ap()[s])
                nc.sync.dma_start(X04[0:3, :], cloudT_d.ap()[s])
                nc.sync.dma_start(cR[:], cloudR_d.ap()[s])
                nc.sync.dma_start(nR[:], noiseR_d.ap()[s])

                # ---- std (two-pass, ddof=1) ----
                red = sp.tile([P, 1], fp32, tag="red")
                nc.vector.tensor_reduce(red[:], cR[:], axis=AX.X, op=OP.add)
                pm = psB.tile([1, 1], fp32, tag="pm")
                nc.tensor.matmul(pm[:], red[:], ones_128x1[:])
                negmean = sp.tile([1, 1], fp32, tag="negmean")
                nc.scalar.activation(negmean[:], pm[:], AF.Identity,
                                     bias=0.0, scale=-1.0 / (N * D))
                negmeanb = sp.tile([P, 1], fp32, tag="negmeanb")
                nc.gpsimd.partition_broadcast(negmeanb[:], negmean[:], channels=P)
                sqdev = sp.tile([P, NG * D], fp32, tag="sqdev")
                nc.scalar.activation(sqdev[:], cR[:], AF.Square,
                                     bias=negmeanb[:], scale=1.0)
                nc.vector.tensor_reduce(red[:], sqdev[:], axis=AX.X, op=OP.add)
                pv = psB.tile([1, 1], fp32, tag="pm")
                nc.tensor.matmul(pv[:], red[:], ones_128x1[:])
                var1 = sp.tile([1, 1], fp32, tag="var1")
                nc.scalar.activation(var1[:], pv[:], AF.Identity,
                                     bias=0.0, scale=1.0 / (N * D - 1))
                std1 = sp.tile([1, 1], fp32, tag="std1")
                nc.scalar.activation(std1[:], var1[:], AF.Sqrt,
                                     bias=0.0, scale=1.0)
                invvar = sp.tile([1, 1], fp32, tag="invvar")
                nc.vector.reciprocal(invvar[:], var1[:])
                invstd = sp.tile([1, 1], fp32, tag="invstd")
                nc.vector.reciprocal(invstd[:], std1[:])
                stdb = sp.tile([P, 1], fp32, tag="stdb")
                nc.gpsimd.partition_broadcast(stdb[:], invstd[:], channels=P)

                # ---- X04 coords = (cloudT / std) * -2 ; csq = cloudT^2 ----
                nc.vector.tensor_scalar(X04[0:3, :], X04[0:3, :],
                                        stdb[32:35, :], -2.0,
                                        op0=OP.mult, op1=OP.mult)
                nc.scalar.activation(csq[:], csq[:], AF.Square,
                                     bias=0.0, scale=1.0)
                for tcol in range(4):
                    pyn = psB.tile([1, 512], fp32, tag="pyn")
                    nc.tensor.matmul(pyn[:], onescol[64:67, :],
                                     csq[:, 512 * tcol:512 * (tcol + 1)])
                    ynsb = dp.tile([1, 512], fp32, tag="eqd")
                    nc.scalar.activation(ynsb[:], pyn[:], AF.Identity,
                                         bias=0.0, scale=invvar[:])
                    nc.sync.dma_start(X04[3:4, 512 * tcol:512 * (tcol + 1)],
                                      ynsb[:])

                # ---- xn per row-group ----
                nsq = sp.tile([P, NG * D], fp32, tag="sqdev")
                nc.scalar.activation(nsq[:], nR[:], AF.Square, bias=0.0, scale=1.0)
                xn = sp.tile([P, NG], fp32, tag="xn")
                nc.vector.tensor_reduce(
                    xn[:], nsq[:].rearrange("p (g d) -> p g d", d=D),
                    axis=AX.X, op=OP.add)
                negxn = sp.tile([P, NG], fp32, tag="negxn")
                nc.vector.tensor_scalar(negxn[:], xn[:], -1.0, None, op0=OP.mult)

                # ---- V0 build fused with iteration-0 scan ----
                V0 = v0pool.tile([P, NG * N], fp32, tag="v0all")
                TOP8 = sp.tile([P, NG * 8], fp32, tag="top8")
                IDX8 = sp.tile([P, NG * 8], u16, tag="idx8")
                for g in range(NG):
                    for tcol in range(4):
                        ps = psA.tile([P, 512], fp32, tag="ps")
                        nc.tensor.matmul(ps[:], nT[:, P * g:P * (g + 1)],
                                         X04[:, 512 * tcol:512 * (tcol + 1)])
                        nc.scalar.activation(
                            V0[:, g * N + 512 * tcol: g * N + 512 * (tcol + 1)],
                            ps[:], AF.Identity, bias=negxn[:, g:g + 1], scale=-1.0)
                    v0g = V0[:, g * N:(g + 1) * N]
                    nc.vector.max(TOP8[:, 8 * g:8 * (g + 1)], v0g)
                    nc.vector.max_index(IDX8[:, 8 * g:8 * (g + 1)],
                                        TOP8[:, 8 * g:8 * (g + 1)], v0g)

                t8v = TOP8[:].rearrange("p (g k) -> p g k", k=8)
                i8v = IDX8[:].rearrange("p (g k) -> p g k", k=8)

                # ---- candidate cache (frozen at it0) + iteration state ----
                M1 = sp.tile([P, NG], fp32, tag="m1")
                M2 = sp.tile([P, NG], fp32, tag="m2")
                JSELU = sp.tile([P, NG], u16, tag="jselu")
                JF = sp.tile([P, NG], fp32, tag="jf")
                BOUND = sp.tile([P, NG], fp32, tag="bound")
                nc.vector.tensor_copy(M1[:], t8v[:, :, 0])
                nc.vector.tensor_copy(M2[:], t8v[:, :, 1])
                nc.vector.tensor_copy(JSELU[:], i8v[:, :, 0])
                nc.vector.tensor_copy(JF[:], JSELU[:])
                nc.vector.tensor_copy(BOUND[:], t8v[:, :, 7])
                CIDX16 = IDX8[:].bitcast(i16)

                PBC = dp.tile([P, N], fp32, tag="pbc")
                nc.vector.memset(PBC[:], 0.0)
                BIDF = sp.tile([P, NG], fp32, tag="bidf")

                for it in range(5):
                    last = (it == 4)
                    if it > 0:
                        # ---- fast path: candidate revalidation ----
                        GOUT = dp.tile([P, N], fp32, tag="vps", bufs=2)
                        nc.gpsimd.ap_gather(
                            GOUT[:].rearrange("p (n d) -> p n d", d=1),
                            PBC[:].rearrange("p (n d) -> p n d", d=1),
                            CIDX16, channels=P, num_elems=N, d=1,
                            num_idxs=2048)
                        SCR = dp.tile([P, N], fp32, tag="vps", bufs=2)
                        gv = GOUT[:].rearrange("p (s j) -> p s j", j=16)
                        dv = DIAG[:].unsqueeze(1).broadcast_to([P, NG * 8, 16])
                        sv = SCR[:].rearrange("p (s j) -> p s j", j=16)
                        nc.vector.tensor_tensor(sv, gv, dv, op=OP.mult)
                        PRG = sp.tile([P, NG * 8], fp32, tag="prg")
                        nc.vector.tensor_reduce(PRG[:], sv, axis=AX.X,
                                                op=OP.add)
                        CANDV = sp.tile([P, NG * 8], fp32, tag="candv")
                        nc.vector.tensor_tensor(CANDV[:], TOP8[:], PRG[:],
                                                op=OP.subtract)
                        cv = CANDV[:].rearrange("p (g k) -> p g k", k=8)
                        nc.vector.tensor_reduce(M1[:], cv, axis=AX.X, op=OP.max)
                        EQ1 = sp.tile([P, NG * 8], u16, tag="eq1")
                        e1v = EQ1[:].rearrange("p (g k) -> p g k", k=8)
                        m1b = M1[:].unsqueeze(2).broadcast_to([P, NG, 8])
                        nc.vector.tensor_tensor(e1v, cv, m1b, op=OP.is_equal)
                        SELI = sp.tile([P, NG * 8], u16, tag="seli")
                        nc.vector.select(SELI[:], EQ1[:], IDX8[:], U16MAX[:])
                        nc.vector.tensor_reduce(
                            JSELU[:],
                            SELI[:].rearrange("p (g k) -> p g k", k=8),
                            axis=AX.X, op=OP.min)
                        EQJ = sp.tile([P, NG * 8], u16, tag="eqj")
                        ejv = EQJ[:].rearrange("p (g k) -> p g k", k=8)
                        jb = JSELU[:].unsqueeze(2).broadcast_to([P, NG, 8])
                        nc.vector.tensor_tensor(
                            ejv, IDX8[:].rearrange("p (g k) -> p g k", k=8),
                            jb, op=OP.is_equal)
                        CV2 = sp.tile([P, NG * 8], fp32, tag="cv2")
                        nc.vector.select(CV2[:], EQJ[:], NEGB[:], CANDV[:])
                        nc.vector.tensor_reduce(
                            M2[:], CV2[:].rearrange("p (g k) -> p g k", k=8),
                            axis=AX.X, op=OP.max)
                        nc.vector.tensor_copy(JF[:], JSELU[:])

                        # ---- validity + conditional rescans ----
                        INVC = sp.tile([P, NG + 1], i32, tag="invc")
                        nc.vector.tensor_tensor(INVC[:, 0:NG], M2[:], BOUND[:],
                                                op=OP.is_lt)
                        nc.vector.tensor_reduce(
                            INVC[:, NG:NG + 1],
                            INVC[:, 0:NG].rearrange("p (o g) -> p o g", o=1),
                            axis=AX.X, op=OP.max)
                        INVR = sp.tile([P, NG + 1], i32, tag="invr")
                        nc.gpsimd.partition_all_reduce(
                            INVR[:], INVC[:], channels=P,
                            reduce_op=bass_isa.ReduceOp.max)

                        TOP8S = sp.tile([P, 8], fp32, tag="top8s")
                        IDX8S = sp.tile([P, 8], u16, tag="idx8s")

                        def rescan(g):
                            VpS = dp.tile([P, N], fp32, tag="vps", bufs=2)
                            nc.vector.tensor_tensor(
                                VpS[:], V0[:, g * N:(g + 1) * N], PBC[:],
                                op=OP.subtract)
                            nc.vector.max(TOP8S[:], VpS[:])
                            nc.vector.max_index(IDX8S[:], TOP8S[:], VpS[:])
                            nc.vector.tensor_copy(M1[:, g:g + 1], TOP8S[:, 0:1])
                            nc.vector.tensor_copy(M2[:, g:g + 1], TOP8S[:, 1:2])
                            nc.vector.tensor_copy(JSELU[:, g:g + 1],
                                                  IDX8S[:, 0:1])
                            nc.vector.tensor_copy(JF[:, g:g + 1],
                                                  IDX8S[:, 0:1])

                        if FASTPATH:
                            ra = nc.vector.value_load(INVR[0:1, NG:NG + 1])
                            with tc.If(ra > 0):
                                for g in range(NG):
                                    rg = nc.vector.value_load(
                                        INVR[0:1, g:g + 1])
                                    with tc.If(rg > 0):
                                        rescan(g)
                        else:
                            for g in range(NG):
                                rescan(g)

                    if last:
                        break

                    # ================= bid + dedup + scatter =================
                    nc.vector.tensor_tensor(BIDF[:], M1[:], M2[:],
                                            op=OP.subtract)
                    nc.vector.tensor_scalar(BIDF[:], BIDF[:], float(EPS), None,
                                            op0=OP.add)

                    ja = JF[:].unsqueeze(2).broadcast_to([P, NG, NG])
                    jb2 = JF[:].unsqueeze(1).broadcast_to([P, NG, NG])
                    ba = BIDF[:].unsqueeze(2).broadcast_to([P, NG, NG])
                    bb = BIDF[:].unsqueeze(1).broadcast_to([P, NG, NG])
                    dA = sp.tile([P, NG * NG], u16, tag="dA")
                    dB = sp.tile([P, NG * NG], u16, tag="dB")
                    dC = sp.tile([P, NG * NG], u16, tag="dC")
                    dAv = dA[:].rearrange("p (a b) -> p a b", b=NG)
                    dBv = dB[:].rearrange("p (a b) -> p a b", b=NG)
                    dCv = dC[:].rearrange("p (a b) -> p a b", b=NG)
                    nc.vector.tensor_tensor(dAv, jb2, ja, op=OP.is_equal)
                    nc.vector.tensor_tensor(dBv, bb, ba, op=OP.is_gt)
                    nc.vector.tensor_tensor(dCv, bb, ba, op=OP.is_equal)
                    ltcv = LTC[:].rearrange("p (a b) -> p a b", b=NG)
                    nc.vector.tensor_tensor(dCv, dCv, ltcv, op=OP.mult)
                    nc.vector.tensor_tensor(dBv, dBv, dCv, op=OP.max)
                    nc.vector.tensor_tensor(dAv, dAv, dBv, op=OP.mult)
                    KILL = sp.tile([P, NG], u16, tag="kill")
                    nc.vector.tensor_reduce(KILL[:], dAv, axis=AX.X, op=OP.max)
                    JEFF = sp.tile([P, NG], fp32, tag="jeff")
                    nc.vector.select(JEFF[:], KILL[:], MINUS1[:], JF[:])

                    GEH = sp.tile([P, NG], u16, tag="geh")
                    nc.vector.tensor_scalar(GEH[:], JEFF[:], 1024.0, None,
                                            op0=OP.is_ge)
                    JAf = sp.tile([P, NG], fp32, tag="jaf")
                    JBm = sp.tile([P, NG], fp32, tag="jbm")
                    JBf = sp.tile([P, NG], fp32, tag="jbf")
                    nc.vector.select(JAf[:], GEH[:], MINUS1[:], JEFF[:])
                    nc.vector.tensor_scalar(JBm[:], JEFF[:], -1024.0, None,
                                            op0=OP.add)
                    nc.vector.select(JBf[:], GEH[:], JBm[:], MINUS1[:])
                    JA16 = sp.tile([P, NG], i16, tag="ja16")
                    JB16 = sp.tile([P, NG], i16, tag="jb16")
                    nc.vector.tensor_copy(JA16[:], JAf[:])
                    nc.vector.tensor_copy(JB16[:], JBf[:])

                    bbits = BIDF[:].bitcast(u16).rearrange(
                        "p (k two) -> p k two", two=2)
                    BLO = sp.tile([P, NG], u16, tag="blo")
                    BHI = sp.tile([P, NG], u16, tag="bhi")
                    nc.vector.tensor_copy(BLO[:], bbits[:, :, 0])
                    nc.vector.tensor_copy(BHI[:], bbits[:, :, 1])

                    MHI = dp.tile([P, N], u16, tag="mhi")
                    MLO = dp.tile([P, N], u16, tag="mlo")
                    for half, idxs in ((0, JA16), (1, JB16)):
                        nc.gpsimd.local_scatter(
                            MHI[:, 1024 * half:1024 * (half + 1)], BHI[:],
                            idxs[:], channels=P, num_elems=1024, num_idxs=NG)
                        nc.gpsimd.local_scatter(
                            MLO[:, 1024 * half:1024 * (half + 1)], BLO[:],
                            idxs[:], channels=P, num_elems=1024, num_idxs=NG)
                    CHI = dp.tile([P, N], u16, tag="chi")
                    nc.gpsimd.partition_all_reduce(CHI[:], MHI[:], channels=P,
                                                   reduce_op=bass_isa.ReduceOp.max)
                    EQD = dp.tile([P, N], u16, tag="eqd")
                    nc.vector.tensor_tensor(EQD[:], MHI[:], CHI[:], op=OP.is_equal)
                    SLO = dp.tile([P, N], u16, tag="mhi")
                    nc.vector.tensor_tensor(SLO[:], MLO[:], EQD[:], op=OP.mult)
                    CLO = dp.tile([P, N], u16, tag="mlo")
                    nc.gpsimd.partition_all_reduce(CLO[:], SLO[:], channels=P,
                                                   reduce_op=bass_isa.ReduceOp.max)

                    PB32 = dp.tile([P, N], i32, tag="vps", bufs=2)
                    pnew16 = PB32[:].bitcast(u16).rearrange(
                        "p (n two) -> p n two", two=2)
                    nc.vector.tensor_copy(pnew16[:, :, 0], CLO[:])
                    nc.vector.tensor_copy(pnew16[:, :, 1], CHI[:])
                    GTZ = dp.tile([P, N], u16, tag="eqd")
                    nc.vector.tensor_scalar(GTZ[:], CHI[:], 0, None, op0=OP.is_gt)
                    nc.vector.copy_predicated(PBC[:], GTZ[:],
                                              PB32[:].bitcast(fp32))

                # ================= output =================
                xa = sp.tile([P, NG * D], fp32, tag="xa")
                JS16 = JSELU[:].bitcast(i16)
                for c in range(3):
                    X0C = dp.tile([P, N], fp32, tag="vps", bufs=2)
                    for tcol in range(4):
                        pb = psA.tile([P, 512], fp32, tag="ps")
                        nc.tensor.matmul(pb[:], ECS[32:35, c * P:(c + 1) * P],
                                         X04[0:3, 512 * tcol:512 * (tcol + 1)])
                        nc.scalar.activation(
                            X0C[:, 512 * tcol:512 * (tcol + 1)],
                            pb[:], AF.Identity, bias=0.0, scale=-0.5)
                    GO = dp.tile([P, NG * 16], fp32, tag="eqd")
                    nc.gpsimd.ap_gather(
                        GO[:].rearrange("p (n d) -> p n d", d=1),
                        X0C[:].rearrange("p (n d) -> p n d", d=1),
                        JS16, channels=P, num_elems=N, d=1, num_idxs=NG * 16)
                    SC2 = dp.tile([P, NG * 16], fp32, tag="chi")
                    go_v = GO[:].rearrange("p (s j) -> p s j", j=16)
                    dv2 = DIAG[:].unsqueeze(1).broadcast_to([P, NG, 16])
                    sc2v = SC2[:].rearrange("p (s j) -> p s j", j=16)
                    nc.vector.tensor_tensor(sc2v, go_v, dv2, op=OP.mult)
                    nc.vector.tensor_reduce(
                        xa[:].rearrange("p (g d) -> p g d", d=D)[:, :, c],
                        sc2v, axis=AX.X, op=OP.add)

                tb1 = sp.tile([1, 1], fp32, tag="tb1")
                nc.sync.dma_start(tb1[:], tv_d.ap()[s].unsqueeze(0))
                TB = sp.tile([P, 1], fp32, tag="tbb")
                nc.gpsimd.partition_broadcast(TB[:], tb1[:], channels=P)
                OMT = sp.tile([P, 1], fp32, tag="omt")
                nc.vector.tensor_scalar(OMT[:], TB[:], -1.0, 1.0,
                                        op0=OP.mult, op1=OP.add)
                XT = sp.tile([P, NG * D], fp32, tag="xt")
                NTt = sp.tile([P, NG * D], fp32, tag="ntt")
                VV = sp.tile([P, NG * D], fp32, tag="vv")
                nc.vector.tensor_scalar(XT[:], xa[:], OMT[:], None,
                                        op0=OP.mult)
                nc.vector.tensor_scalar(NTt[:], nR[:], TB[:], None,
                                        op0=OP.mult)
                nc.vector.tensor_tensor(XT[:], XT[:], NTt[:], op=OP.add)
                nc.vector.tensor_tensor(VV[:], nR[:], xa[:], op=OP.subtract)
                nc.sync.dma_start(out_d.ap()[s, 0], XT[:])
                nc.sync.dma_start(out_d.ap()[s, 1], VV[:])

    nc.compile()
    return nc


_NC_CACHE = None


def _get_nc():
    global _NC_CACHE
    if _NC_CACHE is None:
        _NC_CACHE = _build_program()
    return _NC_CACHE


def _host_prep(cloud, noise, t):
    """Build per-core input maps."""
    ltc = np.zeros((P, NG, NG), np.uint16)
    for g in range(NG):
        ltc[:, g, :g] = 1
    ltc = ltc.reshape(P, NG * NG).astype(np.uint16)
    ecs = np.zeros((35, 3 * P), np.float32)
    for c in range(3):
        ecs[32 + c, c * P:(c + 1) * P] = 1.0
    onesrow = np.ones((1, N), np.float32)
    diag = np.zeros((P, 16), np.float32)
    for p in range(P):
        diag[p, p % 16] = 1.0
    in_maps = []
    for c in range(NCORES):
        sidx = [c * SPC + k for k in range(SPC)]
        noiseT = np.stack([noise[s].T for s in sidx]).astype(np.float32)
        cloudT = np.stack([cloud[s].T for s in sidx]).astype(np.float32)
        cloudR = np.stack([
            cloud[s].reshape(NG, P, D).transpose(1, 0, 2).reshape(P, NG * D)
            for s in sidx]).astype(np.float32)
        noiseR = np.stack([
            noise[s].reshape(NG, P, D).transpose(1, 0, 2).reshape(P, NG * D)
            for s in sidx]).astype(np.float32)
        tv = np.array([[t[s]] for s in sidx], np.float32)
        in_maps.append({
            "noiseT": np.ascontiguousarray(noiseT),
            "cloudT": np.ascontiguousarray(cloudT),
            "cloudR": np.ascontiguousarray(cloudR),
            "noiseR": np.ascontiguousarray(noiseR),
            "tv": tv, "ltc": ltc, "ecs": ecs, "onesrow": onesrow,
            "diag": diag,
        })
    return in_maps


def _host_post(results, B):
    out = np.zeros((2, B, N, D), np.float32)
    for c in range(NCORES):
        o = results[c]["out"]  # [SPC, 2, P, NG*D]
        for k in range(SPC):
            s = c * SPC + k
            for which in range(2):
                arr = o[k, which].reshape(P, NG, D).transpose(1, 0, 2)
                out[which, s] = arr.reshape(N, D)
    return out


def kernel(cloud, noise, t):
    from concourse import bass_utils
    cloud = np.asarray(cloud, np.float32)
    noise = np.asarray(noise, np.float32)
    t = np.asarray(t, np.float32)
    nc = _get_nc()
    in_maps = _host_prep(cloud, noise, t)
    res = bass_utils.run_bass_kernel_spmd(nc, in_maps,
                                          core_ids=list(range(NCORES)))
    return _host_post(res.results, cloud.shape[0])


# revision 8
# speedup vs baseline: 1.1224x; 1.1224x over previous
"""Trainium2 Bass kernel for nn_DiffusionModel (auction-matched flow targets).

Self-contained: accepts FULL inputs (cloud [16,2048,3], noise [16,2048,3],
t [16]), shards batch over 8 NeuronCores (2 samples per core), runs the full
5-iteration bijective auction per sample on device, returns [2,16,2048,3].

v2: candidate-cache fast path. Iteration 0 performs the full dense scan
(PE matmul V0 build fused with DVE Max/MaxIndex top-8 per 128-row group).
Iterations 1-4 revalidate each row's top-2 against the frozen iteration-0
top-8 candidate set: since all prices are >= 0 and non-candidate columns
satisfy V0 <= bound (8th-best V0), a row's current top-2 provably stays
within its candidates whenever cand-m2 >= bound. Groups with any invalid
row are rescanned under a data-dependent tc.If branch (DVE-only body).
Candidate prices are fetched per partition with gpsimd ap_gather (block-
shared indices) + a static diagonal-lane mask reduction. Output alignment
also uses ap_gather instead of dense mask-reduce passes.
"""
import numpy as np

P = 128
N = 2048
NG = 16          # row groups per sample (NG * P = N rows)
D = 3
SPC = 2          # samples per core
EPS = 1e-3
NCORES = 8
REPEAT = 1       # benchmark knob: repeat the whole per-core pipeline
FASTPATH = False  # False: rescan every group every iteration (debug)
CHAIN = False    # False: skip candidate-revalidation chain entirely (debug)


def _build_program():
    import concourse.bass as bass
    import concourse.tile as tile
    from concourse import bacc, mybir, bass_isa

    fp32 = mybir.dt.float32
    u16 = mybir.dt.uint16
    i16 = mybir.dt.int16
    i32 = mybir.dt.int32
    OP = mybir.AluOpType
    AX = mybir.AxisListType
    AF = mybir.ActivationFunctionType

    nc = bacc.Bacc("TRN2", target_bir_lowering=False, debug=False,
                   enable_asserts=False)

    # ---- DRAM I/O ----
    noiseT_d = nc.dram_tensor("noiseT", [SPC, 3, N], fp32, kind="ExternalInput")
    cloudT_d = nc.dram_tensor("cloudT", [SPC, 3, N], fp32, kind="ExternalInput")
    cloudR_d = nc.dram_tensor("cloudR", [SPC, P, NG * D], fp32, kind="ExternalInput")
    noiseR_d = nc.dram_tensor("noiseR", [SPC, P, NG * D], fp32, kind="ExternalInput")
    tv_d = nc.dram_tensor("tv", [SPC, 1], fp32, kind="ExternalInput")
    ltc_d = nc.dram_tensor("ltc", [P, NG * NG], u16, kind="ExternalInput")
    ecs_d = nc.dram_tensor("ecs", [35, 3 * P], fp32, kind="ExternalInput")
    onesrow_d = nc.dram_tensor("onesrow", [1, N], fp32, kind="ExternalInput")
    diag_d = nc.dram_tensor("diag", [P, 16], fp32, kind="ExternalInput")
    out_d = nc.dram_tensor("out", [SPC, 2, P, NG * D], fp32, kind="ExternalOutput")

    with tile.TileContext(nc) as tc:
        with (
            tc.tile_pool(name="v0", bufs=1) as v0pool,
            tc.tile_pool(name="dense", bufs=1) as dp,
            tc.tile_pool(name="small", bufs=1) as sp,
            tc.tile_pool(name="psA", bufs=6, space="PSUM") as psA,
            tc.tile_pool(name="psB", bufs=1, space="PSUM") as psB,
        ):
            # ---- constants (persist across both samples) ----
            LTC = sp.tile([P, NG * NG], u16, tag="ltc")
            nc.sync.dma_start(LTC[:], ltc_d.ap())
            DIAG = sp.tile([P, 16], fp32, tag="diag")
            nc.sync.dma_start(DIAG[:], diag_d.ap())
            ones_128x1 = sp.tile([P, 1], fp32, tag="ones_128x1")
            onescol = sp.tile([67, 1], fp32, tag="onescol")
            MINUS1 = sp.tile([P, NG], fp32, tag="minus1")
            NEGB = sp.tile([P, NG * 8], fp32, tag="negb")
            U16MAX = sp.tile([P, NG * 8], u16, tag="u16max")
            ECS = sp.tile([35, 3 * P], fp32, tag="ecs")
            nc.sync.dma_start(ECS[:], ecs_d.ap())
            nc.vector.memset(ones_128x1[:], 1.0)
            nc.vector.memset(onescol[:], 0.0)
            nc.vector.memset(onescol[64:67, :], 1.0)
            nc.vector.memset(MINUS1[:], -1.0)
            nc.vector.memset(NEGB[:], -3.0e38)
            nc.vector.memset(U16MAX[:], 65535)

            for s in [s for _ in range(REPEAT) for s in range(SPC)]:
                # ================= prep =================
                strip1 = sp.tile([P, N], fp32, tag="strip1")
                strip2 = sp.tile([P, N], fp32, tag=f"strip2_{s % 2}")
                nT = strip1[32:36, :]
                csq = strip1[64:67, :]
                X04 = strip2[32:36, :]
                cR = sp.tile([P, NG * D], fp32, tag="cR")
                nR = sp.tile([P, NG * D], fp32, tag="nR")
                nc.sync.dma_start(nT[0:3, :], noiseT_d.ap()[s])
                nc.sync.dma_start(nT[3:4, :], onesrow_d.ap())
                nc.sync.dma_start(csq[:], cloudT_d.ap()[s])
                nc.sync.dma_start(X04[0:3, :], cloudT_d.ap()[s])
                nc.sync.dma_start(cR[:], cloudR_d.ap()[s])
                nc.sync.dma_start(nR[:], noiseR_d.ap()[s])

                # ---- std (two-pass, ddof=1) ----
                red = sp.tile([P, 1], fp32, tag="red")
                nc.vector.tensor_reduce(red[:], cR[:], axis=AX.X, op=OP.add)
                pm = psB.tile([1, 1], fp32, tag="pm")
                nc.tensor.matmul(pm[:], red[:], ones_128x1[:])
                negmean = sp.tile([1, 1], fp32, tag="negmean")
                nc.scalar.activation(negmean[:], pm[:], AF.Identity,
                                     bias=0.0, scale=-1.0 / (N * D))
                negmeanb = sp.tile([P, 1], fp32, tag="negmeanb")
                nc.gpsimd.partition_broadcast(negmeanb[:], negmean[:], channels=P)
                sqdev = sp.tile([P, NG * D], fp32, tag="sqdev")
                nc.scalar.activation(sqdev[:], cR[:], AF.Square,
                                     bias=negmeanb[:], scale=1.0)
                nc.vector.tensor_reduce(red[:], sqdev[:], axis=AX.X, op=OP.add)
                pv = psB.tile([1, 1], fp32, tag="pm")
                nc.tensor.matmul(pv[:], red[:], ones_128x1[:])
                var1 = sp.tile([1, 1], fp32, tag="var1")
                nc.scalar.activation(var1[:], pv[:], AF.Identity,
                                     bias=0.0, scale=1.0 / (N * D - 1))
                std1 = sp.tile([1, 1], fp32, tag="std1")
                nc.scalar.activation(std1[:], var1[:], AF.Sqrt,
                                     bias=0.0, scale=1.0)
                invvar = sp.tile([1, 1], fp32, tag="invvar")
                nc.vector.reciprocal(invvar[:], var1[:])
                invstd = sp.tile([1, 1], fp32, tag="invstd")
                nc.vector.reciprocal(invstd[:], std1[:])
                stdb = sp.tile([P, 1], fp32, tag="stdb")
                nc.gpsimd.partition_broadcast(stdb[:], invstd[:], channels=P)

                # ---- X04 coords = (cloudT / std) * -2 ; csq = cloudT^2 ----
                nc.vector.tensor_scalar(X04[0:3, :], X04[0:3, :],
                                        stdb[32:35, :], -2.0,
                                        op0=OP.mult, op1=OP.mult)
                nc.scalar.activation(csq[:], csq[:], AF.Square,
                                     bias=0.0, scale=1.0)
                for tcol in range(4):
                    pyn = psB.tile([1, 512], fp32, tag="pyn")
                    nc.tensor.matmul(pyn[:], onescol[64:67, :],
                                     csq[:, 512 * tcol:512 * (tcol + 1)])
                    ynsb = dp.tile([1, 512], fp32, tag="eqd")
                    nc.scalar.activation(ynsb[:], pyn[:], AF.Identity,
                                         bias=0.0, scale=invvar[:])
                    nc.sync.dma_start(X04[3:4, 512 * tcol:512 * (tcol + 1)],
                                      ynsb[:])

                # ---- xn per row-group ----
                nsq = sp.tile([P, NG * D], fp32, tag="sqdev")
                nc.scalar.activation(nsq[:], nR[:], AF.Square, bias=0.0, scale=1.0)
                xn = sp.tile([P, NG], fp32, tag="xn")
                nc.vector.tensor_reduce(
                    xn[:], nsq[:].rearrange("p (g d) -> p g d", d=D),
                    axis=AX.X, op=OP.add)
                negxn = sp.tile([P, NG], fp32, tag="negxn")
                nc.vector.tensor_scalar(negxn[:], xn[:], -1.0, None, op0=OP.mult)

                # ---- V0 build fused with iteration-0 scan ----
                V0 = v0pool.tile([P, NG * N], fp32, tag="v0all")
                TOP8 = sp.tile([P, NG * 8], fp32, tag="top8")
                IDX8 = sp.tile([P, NG * 8], u16, tag="idx8")
                for g in range(NG):
                    for tcol in range(4):
                        ps = psA.tile([P, 512], fp32, tag="ps")
                        nc.tensor.matmul(ps[:], nT[:, P * g:P * (g + 1)],
                                         X04[:, 512 * tcol:512 * (tcol + 1)])
                        nc.scalar.activation(
                            V0[:, g * N + 512 * tcol: g * N + 512 * (tcol + 1)],
                            ps[:], AF.Identity, bias=negxn[:, g:g + 1], scale=-1.0)
                    v0g = V0[:, g * N:(g + 1) * N]
                    nc.vector.max(TOP8[:, 8 * g:8 * (g + 1)], v0g)
                    nc.vector.max_index(IDX8[:, 8 * g:8 * (g + 1)],
                                        TOP8[:, 8 * g:8 * (g + 1)], v0g)

                t8v = TOP8[:].rearrange("p (g k) -> p g k", k=8)
                i8v = IDX8[:].rearrange("p (g k) -> p g k", k=8)

                # ---- candidate cache (frozen at it0) + iteration state ----
                M1 = sp.tile([P, NG], fp32, tag="m1")
                M2 = sp.tile([P, NG], fp32, tag="m2")
                JSELU = sp.tile([P, NG], u16, tag="jselu")
                JF = sp.tile([P, NG], fp32, tag="jf")
                BOUND = sp.tile([P, NG], fp32, tag="bound")
                nc.vector.tensor_copy(M1[:], t8v[:, :, 0])
                nc.vector.tensor_copy(M2[:], t8v[:, :, 1])
                nc.vector.tensor_copy(JSELU[:], i8v[:, :, 0])
                nc.vector.tensor_copy(JF[:], JSELU[:])
                nc.vector.tensor_copy(BOUND[:], t8v[:, :, 7])
                CIDX16 = IDX8[:].bitcast(i16)

                PBC = dp.tile([P, N], fp32, tag="pbc")
                nc.vector.memset(PBC[:], 0.0)
                BIDF = sp.tile([P, NG], fp32, tag="bidf")

                for it in range(5):
                    last = (it == 4)
                    if it > 0 and CHAIN:
                        # ---- fast path: candidate revalidation ----
                        GOUT = dp.tile([P, N], fp32, tag="vps", bufs=2)
                        nc.gpsimd.ap_gather(
                            GOUT[:].rearrange("p (n d) -> p n d", d=1),
                            PBC[:].rearrange("p (n d) -> p n d", d=1),
                            CIDX16, channels=P, num_elems=N, d=1,
                            num_idxs=2048)
                        SCR = dp.tile([P, N], fp32, tag="vps", bufs=2)
                        gv = GOUT[:].rearrange("p (s j) -> p s j", j=16)
                        dv = DIAG[:].unsqueeze(1).broadcast_to([P, NG * 8, 16])
                        sv = SCR[:].rearrange("p (s j) -> p s j", j=16)
                        nc.vector.tensor_tensor(sv, gv, dv, op=OP.mult)
                        PRG = sp.tile([P, NG * 8], fp32, tag="prg")
                        nc.vector.tensor_reduce(PRG[:], sv, axis=AX.X,
                                                op=OP.add)
                        CANDV = sp.tile([P, NG * 8], fp32, tag="candv")
                        nc.vector.tensor_tensor(CANDV[:], TOP8[:], PRG[:],
                                                op=OP.subtract)
                        cv = CANDV[:].rearrange("p (g k) -> p g k", k=8)
                        nc.vector.tensor_reduce(M1[:], cv, axis=AX.X, op=OP.max)
                        EQ1 = sp.tile([P, NG * 8], u16, tag="eq1")
                        e1v = EQ1[:].rearrange("p (g k) -> p g k", k=8)
                        m1b = M1[:].unsqueeze(2).broadcast_to([P, NG, 8])
                        nc.vector.tensor_tensor(e1v, cv, m1b, op=OP.is_equal)
                        SELI = sp.tile([P, NG * 8], u16, tag="seli")
                        nc.vector.select(SELI[:], EQ1[:], IDX8[:], U16MAX[:])
                        nc.vector.tensor_reduce(
                            JSELU[:],
                            SELI[:].rearrange("p (g k) -> p g k", k=8),
                            axis=AX.X, op=OP.min)
                        EQJ = sp.tile([P, NG * 8], u16, tag="eqj")
                        ejv = EQJ[:].rearrange("p (g k) -> p g k", k=8)
                        jb = JSELU[:].unsqueeze(2).broadcast_to([P, NG, 8])
                        nc.vector.tensor_tensor(
                            ejv, IDX8[:].rearrange("p (g k) -> p g k", k=8),
                            jb, op=OP.is_equal)
                        CV2 = sp.tile([P, NG * 8], fp32, tag="cv2")
                        nc.vector.select(CV2[:], EQJ[:], NEGB[:], CANDV[:])
                        nc.vector.tensor_reduce(
                            M2[:], CV2[:].rearrange("p (g k) -> p g k", k=8),
                            axis=AX.X, op=OP.max)
                        nc.vector.tensor_copy(JF[:], JSELU[:])

                        # ---- validity + conditional rescans ----
                        INVC = sp.tile([P, NG + 1], i32, tag="invc")
                        nc.vector.tensor_tensor(INVC[:, 0:NG], M2[:], BOUND[:],
                                                op=OP.is_lt)
                        nc.vector.tensor_reduce(
                            INVC[:, NG:NG + 1],
                            INVC[:, 0:NG].rearrange("p (o g) -> p o g", o=1),
                            axis=AX.X, op=OP.max)
                        INVR = sp.tile([P, NG + 1], i32, tag="invr")
                        nc.gpsimd.partition_all_reduce(
                            INVR[:], INVC[:], channels=P,
                            reduce_op=bass_isa.ReduceOp.max)

                    if it > 0:
                        TOP8S = sp.tile([P, 8], fp32, tag="top8s")
                        IDX8S = sp.tile([P, 8], u16, tag="idx8s")
                        if it == 1:
                            nc.vector.memset(TOP8S[:], 0.0)
                            nc.vector.memset(IDX8S[:], 0)

                        def rescan(g):
                            VpS = dp.tile([P, N], fp32, tag="vps", bufs=2)
                            nc.vector.tensor_tensor(
                                VpS[:], V0[:, g * N:(g + 1) * N], PBC[:],
                                op=OP.subtract)
                            nc.vector.max(TOP8S[:], VpS[:])
                            nc.vector.max_index(IDX8S[:], TOP8S[:], VpS[:])
                            nc.vector.tensor_copy(M1[:, g:g + 1], TOP8S[:, 0:1])
                            nc.vector.tensor_copy(M2[:, g:g + 1], TOP8S[:, 1:2])
                            nc.vector.tensor_copy(JSELU[:, g:g + 1],
                                                  IDX8S[:, 0:1])
                            nc.vector.tensor_copy(JF[:, g:g + 1],
                                                  IDX8S[:, 0:1])

                        if FASTPATH:
                            ra = nc.vector.value_load(INVR[0:1, NG:NG + 1])
                            with tc.If(ra > 0):
                                for g in range(NG):
                                    rg = nc.vector.value_load(
                                        INVR[0:1, g:g + 1])
                                    with tc.If(rg > 0):
                                        rescan(g)
                        else:
                            for g in range(NG):
                                rescan(g)

                    if last:
                        break

                    # ================= bid + dedup + scatter =================
                    nc.vector.tensor_tensor(BIDF[:], M1[:], M2[:],
                                            op=OP.subtract)
                    nc.vector.tensor_scalar(BIDF[:], BIDF[:], float(EPS), None,
                                            op0=OP.add)

                    ja = JF[:].unsqueeze(2).broadcast_to([P, NG, NG])
                    jb2 = JF[:].unsqueeze(1).broadcast_to([P, NG, NG])
                    ba = BIDF[:].unsqueeze(2).broadcast_to([P, NG, NG])
                    bb = BIDF[:].unsqueeze(1).broadcast_to([P, NG, NG])
                    dA = sp.tile([P, NG * NG], u16, tag="dA")
                    dB = sp.tile([P, NG * NG], u16, tag="dB")
                    dC = sp.tile([P, NG * NG], u16, tag="dC")
                    dAv = dA[:].rearrange("p (a b) -> p a b", b=NG)
                    dBv = dB[:].rearrange("p (a b) -> p a b", b=NG)
                    dCv = dC[:].rearrange("p (a b) -> p a b", b=NG)
                    nc.vector.tensor_tensor(dAv, jb2, ja, op=OP.is_equal)
                    nc.vector.tensor_tensor(dBv, bb, ba, op=OP.is_gt)
                    nc.vector.tensor_tensor(dCv, bb, ba, op=OP.is_equal)
                    ltcv = LTC[:].rearrange("p (a b) -> p a b", b=NG)
                    nc.vector.tensor_tensor(dCv, dCv, ltcv, op=OP.mult)
                    nc.vector.tensor_tensor(dBv, dBv, dCv, op=OP.max)
                    nc.vector.tensor_tensor(dAv, dAv, dBv, op=OP.mult)
                    KILL = sp.tile([P, NG], u16, tag="kill")
                    nc.vector.tensor_reduce(KILL[:], dAv, axis=AX.X, op=OP.max)
                    JEFF = sp.tile([P, NG], fp32, tag="jeff")
                    nc.vector.select(JEFF[:], KILL[:], MINUS1[:], JF[:])

                    GEH = sp.tile([P, NG], u16, tag="geh")
                    nc.vector.tensor_scalar(GEH[:], JEFF[:], 1024.0, None,
                                            op0=OP.is_ge)
                    JAf = sp.tile([P, NG], fp32, tag="jaf")
                    JBm = sp.tile([P, NG], fp32, tag="jbm")
                    JBf = sp.tile([P, NG], fp32, tag="jbf")
                    nc.vector.select(JAf[:], GEH[:], MINUS1[:], JEFF[:])
                    nc.vector.tensor_scalar(JBm[:], JEFF[:], -1024.0, None,
                                            op0=OP.add)
                    nc.vector.select(JBf[:], GEH[:], JBm[:], MINUS1[:])
                    JA16 = sp.tile([P, NG], i16, tag="ja16")
                    JB16 = sp.tile([P, NG], i16, tag="jb16")
                    nc.vector.tensor_copy(JA16[:], JAf[:])
                    nc.vector.tensor_copy(JB16[:], JBf[:])

                    bbits = BIDF[:].bitcast(u16).rearrange(
                        "p (k two) -> p k two", two=2)
                    BLO = sp.tile([P, NG], u16, tag="blo")
                    BHI = sp.tile([P, NG], u16, tag="bhi")
                    nc.vector.tensor_copy(BLO[:], bbits[:, :, 0])
                    nc.vector.tensor_copy(BHI[:], bbits[:, :, 1])

                    MHI = dp.tile([P, N], u16, tag="mhi")
                    MLO = dp.tile([P, N], u16, tag="mlo")
                    for half, idxs in ((0, JA16), (1, JB16)):
                        nc.gpsimd.local_scatter(
                            MHI[:, 1024 * half:1024 * (half + 1)], BHI[:],
                            idxs[:], channels=P, num_elems=1024, num_idxs=NG)
                        nc.gpsimd.local_scatter(
                            MLO[:, 1024 * half:1024 * (half + 1)], BLO[:],
                            idxs[:], channels=P, num_elems=1024, num_idxs=NG)
                    CHI = dp.tile([P, N], u16, tag="chi")
                    nc.gpsimd.partition_all_reduce(CHI[:], MHI[:], channels=P,
                                                   reduce_op=bass_isa.ReduceOp.max)
                    EQD = dp.tile([P, N], u16, tag="eqd")
                    nc.vector.tensor_tensor(EQD[:], MHI[:], CHI[:], op=OP.is_equal)
                    SLO = dp.tile([P, N], u16, tag="mhi")
                    nc.vector.tensor_tensor(SLO[:], MLO[:], EQD[:], op=OP.mult)
                    CLO = dp.tile([P, N], u16, tag="mlo")
                    nc.gpsimd.partition_all_reduce(CLO[:], SLO[:], channels=P,
                                                   reduce_op=bass_isa.ReduceOp.max)

                    PB32 = dp.tile([P, N], i32, tag="vps", bufs=2)
                    pnew16 = PB32[:].bitcast(u16).rearrange(
                        "p (n two) -> p n two", two=2)
                    nc.vector.tensor_copy(pnew16[:, :, 0], CLO[:])
                    nc.vector.tensor_copy(pnew16[:, :, 1], CHI[:])
                    GTZ = dp.tile([P, N], u16, tag="eqd")
                    nc.vector.tensor_scalar(GTZ[:], CHI[:], 0, None, op0=OP.is_gt)
                    nc.vector.copy_predicated(PBC[:], GTZ[:],
                                              PB32[:].bitcast(fp32))

                # ================= output =================
                xa = sp.tile([P, NG * D], fp32, tag="xa")
                JS16 = JSELU[:].bitcast(i16)
                for c in range(3):
                    X0C = dp.tile([P, N], fp32, tag="vps", bufs=2)
                    for tcol in range(4):
                        pb = psA.tile([P, 512], fp32, tag="ps")
                        nc.tensor.matmul(pb[:], ECS[32:35, c * P:(c + 1) * P],
                                         X04[0:3, 512 * tcol:512 * (tcol + 1)])
                        nc.scalar.activation(
                            X0C[:, 512 * tcol:512 * (tcol + 1)],
                            pb[:], AF.Identity, bias=0.0, scale=-0.5)
                    GO = dp.tile([P, NG * 16], fp32, tag="eqd")
                    nc.gpsimd.ap_gather(
                        GO[:].rearrange("p (n d) -> p n d", d=1),
                        X0C[:].rearrange("p (n d) -> p n d", d=1),
                        JS16, channels=P, num_elems=N, d=1, num_idxs=NG * 16)
                    SC2 = dp.tile([P, NG * 16], fp32, tag="chi")
                    go_v = GO[:].rearrange("p (s j) -> p s j", j=16)
                    dv2 = DIAG[:].unsqueeze(1).broadcast_to([P, NG, 16])
                    sc2v = SC2[:].rearrange("p (s j) -> p s j", j=16)
                    nc.vector.tensor_tensor(sc2v, go_v, dv2, op=OP.mult)
                    nc.vector.tensor_reduce(
                        xa[:].rearrange("p (g d) -> p g d", d=D)[:, :, c],
                        sc2v, axis=AX.X, op=OP.add)

                tb1 = sp.tile([1, 1], fp32, tag="tb1")
                nc.sync.dma_start(tb1[:], tv_d.ap()[s].unsqueeze(0))
                TB = sp.tile([P, 1], fp32, tag="tbb")
                nc.gpsimd.partition_broadcast(TB[:], tb1[:], channels=P)
                OMT = sp.tile([P, 1], fp32, tag="omt")
                nc.vector.tensor_scalar(OMT[:], TB[:], -1.0, 1.0,
                                        op0=OP.mult, op1=OP.add)
                XT = sp.tile([P, NG * D], fp32, tag="xt")
                NTt = sp.tile([P, NG * D], fp32, tag="ntt")
                VV = sp.tile([P, NG * D], fp32, tag="vv")
                nc.vector.tensor_scalar(XT[:], xa[:], OMT[:], None,
                                        op0=OP.mult)
                nc.vector.tensor_scalar(NTt[:], nR[:], TB[:], None,
                                        op0=OP.mult)
                nc.vector.tensor_tensor(XT[:], XT[:], NTt[:], op=OP.add)
                nc.vector.tensor_tensor(VV[:], nR[:], xa[:], op=OP.subtract)
                nc.sync.dma_start(out_d.ap()[s, 0], XT[:])
                nc.sync.dma_start(out_d.ap()[s, 1], VV[:])

    nc.compile()
    return nc


_NC_CACHE = None


def _get_nc():
    global _NC_CACHE
    if _NC_CACHE is None:
        _NC_CACHE = _build_program()
    return _NC_CACHE


def _host_prep(cloud, noise, t):
    """Build per-core input maps."""
    ltc = np.zeros((P, NG, NG), np.uint16)
    for g in range(NG):
        ltc[:, g, :g] = 1
    ltc = ltc.reshape(P, NG * NG).astype(np.uint16)
    ecs = np.zeros((35, 3 * P), np.float32)
    for c in range(3):
        ecs[32 + c, c * P:(c + 1) * P] = 1.0
    onesrow = np.ones((1, N), np.float32)
    diag = np.zeros((P, 16), np.float32)
    for p in range(P):
        diag[p, p % 16] = 1.0
    in_maps = []
    for c in range(NCORES):
        sidx = [c * SPC + k for k in range(SPC)]
        noiseT = np.stack([noise[s].T for s in sidx]).astype(np.float32)
        cloudT = np.stack([cloud[s].T for s in sidx]).astype(np.float32)
        cloudR = np.stack([
            cloud[s].reshape(NG, P, D).transpose(1, 0, 2).reshape(P, NG * D)
            for s in sidx]).astype(np.float32)
        noiseR = np.stack([
            noise[s].reshape(NG, P, D).transpose(1, 0, 2).reshape(P, NG * D)
            for s in sidx]).astype(np.float32)
        tv = np.array([[t[s]] for s in sidx], np.float32)
        in_maps.append({
            "noiseT": np.ascontiguousarray(noiseT),
            "cloudT": np.ascontiguousarray(cloudT),
            "cloudR": np.ascontiguousarray(cloudR),
            "noiseR": np.ascontiguousarray(noiseR),
            "tv": tv, "ltc": ltc, "ecs": ecs, "onesrow": onesrow,
            "diag": diag,
        })
    return in_maps


def _host_post(results, B):
    out = np.zeros((2, B, N, D), np.float32)
    for c in range(NCORES):
        o = results[c]["out"]  # [SPC, 2, P, NG*D]
        for k in range(SPC):
            s = c * SPC + k
            for which in range(2):
                arr = o[k, which].reshape(P, NG, D).transpose(1, 0, 2)
                out[which, s] = arr.reshape(N, D)
    return out


def kernel(cloud, noise, t):
    from concourse import bass_utils
    cloud = np.asarray(cloud, np.float32)
    noise = np.asarray(noise, np.float32)
    t = np.asarray(t, np.float32)
    nc = _get_nc()
    in_maps = _host_prep(cloud, noise, t)
    res = bass_utils.run_bass_kernel_spmd(nc, in_maps,
                                          core_ids=list(range(NCORES)))
    return _host_post(res.results, cloud.shape[0])
